# revision 20
# baseline (speedup 1.0000x reference)
"""Trainium2 Bass kernel for nn_DHP_1314259992584 (GNN message passing).

Computation (see reference):
  x0 = emb
  loss  = inner_prob(x0) ; x1 = gcn(x0, W1, b1) ; loss += inner_prob(x1)
  x1r = relu(x1)         ; x2 = gcn(x1r, W2, b2); loss += inner_prob(x2)
  out = x0 + 0.5*x1r + x2/3 , loss

Distribution (8 NeuronCores):
  - Node rows sharded: RPC=6272 rows/core (49 windows of 128; N padded to 50176).
  - Edges bucketed by destination window, src indices split at table half
    (25088) so gather indices fit in int16 for dma_gather.
  - segment_sum = selection-matrix matmul accumulated in PSUM per window.
  - Small weights replicated; support1/support2/concat(x0|x1|x2) AllGathered.
  - pos/neg hyperedge pooling = the same gather+Sel-matmul machinery over the
    concat table (1536B rows); per-core scalar loss partial, summed on host.
"""

import numpy as np

# ---------------- problem constants (hardcoded; full-size defaults) ---------
N = 50000
D = 128
E = 800000
S = 10000
RATIO = 5
KH = 8            # nodes per hyperedge
NH = 256
NCORES = 8
P = 128


def _derive(n, ncores, n_hyper):
    wpc = ((n + ncores - 1) // ncores + P - 1) // P
    rpc = wpc * P                    # rows per core
    npad = rpc * ncores              # padded node count
    half = npad // 2                 # gather table split point (int16 safety)
    hpc_real = (n_hyper + ncores - 1) // ncores
    lwin = (hpc_real + P - 1) // P   # loss windows per core
    hpc = lwin * P                   # padded hyperedges per core
    return wpc, rpc, npad, half, hpc_real, lwin, hpc


# ---------------------------------------------------------------------------
# Host-side planning: bucket/pad edge lists and hyperedge slot lists.
# ---------------------------------------------------------------------------

def _plan_segments(dst_local, src, val, n_windows, half, ncores_assign):
    """Group (dst_local, src, val) lists of one core by (window, src-half).

    Returns dict (w, h) -> (src_local int64, dmod int64, val f32) arrays."""
    w = dst_local // P
    dmod = dst_local % P
    h = (src >= half).astype(np.int64)
    sloc = src - h * half
    out = {}
    order = np.lexsort((h, w))
    w_s, h_s, sloc_s, dmod_s, val_s = w[order], h[order], sloc[order], dmod[order], val[order]
    keys = w_s * 2 + h_s
    boundaries = np.flatnonzero(np.diff(keys)) + 1
    starts = np.concatenate([[0], boundaries])
    ends = np.concatenate([boundaries, [len(keys)]])
    for a, b in zip(starts, ends):
        if b > a:
            out[(int(w_s[a]), int(h_s[a]))] = (sloc_s[a:b], dmod_s[a:b], val_s[a:b])
    return out


def _pad_tiles(groups_per_core, n_windows, bw):
    """Compute the SPMD-shared tile counts NT[w][h] = max over cores of
    ceil(count/128), and the canonical batch layout.

    Canonical order: for each batch of `bw` windows: for h in (0,1):
    for w in batch: NT[w][h] tiles of 128 slots."""
    nt = np.zeros((n_windows, 2), dtype=np.int64)
    for groups in groups_per_core:
        for (w, h), (sl, dm, vl) in groups.items():
            nt[w, h] = max(nt[w, h], (len(sl) + P - 1) // P)
    nt = np.maximum(nt, 1)  # keep >=1 tile so every window exists in program
    batches = [list(range(b, min(b + bw, n_windows))) for b in range(0, n_windows, bw)]
    return nt, batches


def _emit_streams(groups, nt, batches):
    """Build per-core canonical-order streams.

    Returns (idx0 int16, idx1 int16, meta f32[Ttot*128,2]) where meta rows
    follow global tile order and idxK follows per-half slot order."""
    idx_h = [[], []]
    meta = []
    for batch in batches:
        for h in (0, 1):
            for w in batch:
                k = int(nt[w, h])
                sl, dm, vl = groups.get((w, h), (np.zeros(0, np.int64),) * 2 + (np.zeros(0, np.float32),))
                pad = k * P - len(sl)
                sl = np.concatenate([sl, np.zeros(pad, np.int64)])
                dm = np.concatenate([dm, np.zeros(pad, np.int64)])
                vl = np.concatenate([vl.astype(np.float32), np.zeros(pad, np.float32)])
                idx_h[h].append(sl)
                meta.append(np.stack([dm.astype(np.float32), vl], axis=1))
    idx0 = np.concatenate(idx_h[0]) if idx_h[0] else np.zeros(0, np.int64)
    idx1 = np.concatenate(idx_h[1]) if idx_h[1] else np.zeros(0, np.int64)
    meta = np.concatenate(meta, axis=0)
    assert idx0.max(initial=0) < 32768 and idx1.max(initial=0) < 32768
    return idx0.astype(np.int16), idx1.astype(np.int16), meta.astype(np.float32)


def _wrap_idx(idx):
    """int16 stream -> [128, len/16] SBUF image (16-partition wrap, 8x replicated)."""
    n = len(idx)
    assert n % 16 == 0
    img = idx.reshape(n // 16, 16).T  # [16, n/16]
    return np.tile(img, (8, 1)).copy()  # [128, n/16]


# ---------------------------------------------------------------------------
# Device program
# ---------------------------------------------------------------------------

def _build_program(cfg):
    import concourse.bacc as bacc
    import concourse.bass as bass
    import concourse.tile as tile
    import concourse.mybir as mybir
    from concourse._compat import axon_active
    from concourse.masks import make_identity

    dt = mybir.dt
    AF = mybir.ActivationFunctionType
    OP = mybir.AluOpType

    n_pad, rpc, wpc, half = cfg["npad"], cfg["rpc"], cfg["wpc"], cfg["half"]
    lwin, ncores = cfg["lwin"], cfg["ncores"]
    g_nt, g_batches = cfg["g_nt"], cfg["g_batches"]
    l_nt, l_batches = cfg["l_nt"], cfg["l_batches"]
    g_c0, g_c1 = cfg["g_cols0"], cfg["g_cols1"]
    l_c0, l_c1 = cfg["l_cols0"], cfg["l_cols1"]
    g_T = int(g_nt.sum())
    l_T = int(l_nt.sum())
    CD = 3 * D                       # concat row width
    LG = lwin * CD                   # logits flat length
    LG_T = (LG + P - 1) // P         # bce columns
    use_f32r = cfg.get("use_f32r", False)
    stop_phase = cfg.get("stop_phase", 99)
    mm_dt = dt.float32r if use_f32r else dt.float32

    nc = bacc.Bacc("TRN2", target_bir_lowering=False,
                   debug=False, num_devices=ncores)
    rg = [list(range(ncores))]

    # ---- I/O ----
    emb_shard = nc.dram_tensor("emb_shard", [rpc, D], dt.float32, kind="ExternalInput")
    W1 = nc.dram_tensor("W1", [D, D], dt.float32, kind="ExternalInput")
    b1 = nc.dram_tensor("b1", [D, 1], dt.float32, kind="ExternalInput")
    W2 = nc.dram_tensor("W2", [D, D], dt.float32, kind="ExternalInput")
    b2 = nc.dram_tensor("b2", [D, 1], dt.float32, kind="ExternalInput")
    h_w = nc.dram_tensor("h_w", [D, NH], dt.float32, kind="ExternalInput")
    h_b = nc.dram_tensor("h_b", [NH, 1], dt.float32, kind="ExternalInput")
    p_w = nc.dram_tensor("p_w", [NH, 1], dt.float32, kind="ExternalInput")
    p_b = nc.dram_tensor("p_b", [1, 1], dt.float32, kind="ExternalInput")
    g_idx0 = nc.dram_tensor("g_idx0", [P, g_c0], dt.int16, kind="ExternalInput")
    g_idx1 = nc.dram_tensor("g_idx1", [P, g_c1], dt.int16, kind="ExternalInput")
    g_meta = nc.dram_tensor("g_meta", [g_T * P, 2], dt.float32, kind="ExternalInput")
    l_idx0 = nc.dram_tensor("l_idx0", [P, l_c0], dt.int16, kind="ExternalInput")
    l_idx1 = nc.dram_tensor("l_idx1", [P, l_c1], dt.int16, kind="ExternalInput")
    l_meta = nc.dram_tensor("l_meta", [l_T * P, 2], dt.float32, kind="ExternalInput")
    negsign = nc.dram_tensor("negsign", [P, LG_T], dt.float32, kind="ExternalInput")
    lweight = nc.dram_tensor("lweight", [P, LG_T], dt.float32, kind="ExternalInput")

    out_shard = nc.dram_tensor("out_shard", [rpc, D], dt.float32, kind="ExternalOutput")
    loss_part = nc.dram_tensor("loss_part", [1, 1], dt.float32, kind="ExternalOutput")

    # ---- internal DRAM ----
    s1_shard = nc.dram_tensor("s1_shard", [rpc, D], dt.float32)
    s1_full = nc.dram_tensor("s1_full", [n_pad, D], dt.float32, addr_space="Shared")
    s2_shard = nc.dram_tensor("s2_shard", [rpc, D], dt.float32)
    s2_full = nc.dram_tensor("s2_full", [n_pad, D], dt.float32, addr_space="Shared")
    cc_shard = nc.dram_tensor("cc_shard", [rpc, CD], dt.float32)
    cc_full = nc.dram_tensor("cc_full", [n_pad, CD], dt.float32, addr_space="Shared")
    x1r_stash = nc.dram_tensor("x1r_stash", [rpc, D], dt.float32)
    logits_dram = nc.dram_tensor("logits_dram", [lwin, CD], dt.float32)

    GBW = max(int(sum(g_nt[w, h] for w in b for h in (0, 1))) for b in g_batches)
    LBW = max(int(sum(l_nt[w, h] for w in b for h in (0, 1))) for b in l_batches)

    with tile.TileContext(nc) as tc:
        with (
            tc.tile_pool(name="const", bufs=1) as cpool,
            tc.tile_pool(name="gbuf", bufs=2) as gpool,
            tc.tile_pool(name="lgbuf", bufs=2) as lgpool,
            tc.tile_pool(name="sel", bufs=4) as selpool,
            tc.tile_pool(name="work", bufs=3) as wpool,
            tc.tile_pool(name="small", bufs=4) as spool,
        ):
            # ---- resident constants ----
            ident = cpool.tile([P, P], dt.float32)
            make_identity(nc, ident[:])
            iota_i = cpool.tile([P, P], dt.int32)
            nc.gpsimd.iota(iota_i[:], pattern=[[1, P]], base=0, channel_multiplier=0)
            iota_f = cpool.tile([P, P], dt.float32)
            nc.vector.tensor_copy(iota_f[:], iota_i[:])
            ones_col = cpool.tile([P, 1], dt.float32)
            nc.vector.memset(ones_col[:], 1.0)

            W1_sb = cpool.tile([D, D], dt.float32)
            nc.sync.dma_start(W1_sb[:], W1[:, :])
            W2_sb = cpool.tile([D, D], dt.float32)
            nc.sync.dma_start(W2_sb[:], W2[:, :])
            b1_sb = cpool.tile([D, 1], dt.float32)
            nc.sync.dma_start(b1_sb[:], b1[:, :])
            b2_sb = cpool.tile([D, 1], dt.float32)
            nc.sync.dma_start(b2_sb[:], b2[:, :])
            hw_sb = cpool.tile([D, NH], dt.float32)
            nc.sync.dma_start(hw_sb[:], h_w[:, :])
            hb_sb = cpool.tile([P, 2], dt.float32)
            nc.sync.dma_start(hb_sb[:, 0:1], h_b[0:P, :])
            nc.sync.dma_start(hb_sb[:, 1:2], h_b[P:NH, :])
            pw_sb = cpool.tile([P, 2], dt.float32)
            nc.sync.dma_start(pw_sb[:, 0:1], p_w[0:P, :])
            nc.sync.dma_start(pw_sb[:, 1:2], p_w[P:NH, :])
            pb_sb = cpool.tile([1, 1], dt.float32)
            nc.sync.dma_start(pb_sb[:], p_b[:, :])

            g_idx0_sb = cpool.tile([P, g_c0], dt.int16)
            nc.sync.dma_start(g_idx0_sb[:], g_idx0[:, :])
            g_idx1_sb = cpool.tile([P, g_c1], dt.int16)
            nc.sync.dma_start(g_idx1_sb[:], g_idx1[:, :])
            g_meta_sb = cpool.tile([P, g_T, 2], dt.float32)
            nc.sync.dma_start(g_meta_sb[:],
                              g_meta.ap().rearrange("(t p) c -> p t c", p=P))
            l_idx0_sb = cpool.tile([P, l_c0], dt.int16)
            nc.sync.dma_start(l_idx0_sb[:], l_idx0[:, :])
            l_idx1_sb = cpool.tile([P, l_c1], dt.int16)
            nc.sync.dma_start(l_idx1_sb[:], l_idx1[:, :])
            l_meta_sb = cpool.tile([P, l_T, 2], dt.float32)
            nc.sync.dma_start(l_meta_sb[:],
                              l_meta.ap().rearrange("(t p) c -> p t c", p=P))

            def mm(out, lhsT, rhs, **kw):
                if use_f32r:
                    lhsT = lhsT.bitcast(dt.float32r)
                    rhs = rhs.bitcast(dt.float32r)
                nc.tensor.matmul(out, lhsT, rhs, **kw)

            # ============ phase 1: support1 shard + emb -> concat ============
            gcn_psum = tc.tile_pool(name="gcn_psum", bufs=2, space="PSUM")
            ppool = gcn_psum.__enter__()
            for w in range(wpc):
                rows = slice(w * P, (w + 1) * P)
                emb_t = wpool.tile([P, D], dt.float32, tag="emb_t")
                nc.sync.dma_start(emb_t[:], emb_shard[rows, :])
                nc.sync.dma_start(cc_shard[rows, 0:D], emb_t[:])
                tp = ppool.tile([P, P], dt.float32, tag="tp")
                nc.tensor.transpose(out=tp[:], in_=emb_t[:], identity=ident[:])
                embT = wpool.tile([P, P], dt.float32, tag="embT")
                nc.scalar.activation(embT[:], tp[:], AF.Copy)
                s1p = ppool.tile([P, D], dt.float32, tag="s1p")
                nc.tensor.matmul(s1p[:], lhsT=embT[:], rhs=W1_sb[:])
                s1t = wpool.tile([P, D], dt.float32, tag="s1t")
                nc.scalar.activation(s1t[:], s1p[:], AF.Copy)
                nc.sync.dma_start(s1_shard[rows, :], s1t[:])

            nc.gpsimd.collective_compute(
                "AllGather", OP.bypass, replica_groups=rg,
                ins=[s1_shard.ap().opt()], outs=[s1_full.ap().opt()])

            # ============ GCN layer (shared routine) ============
            def gcn_layer(src_full, bias_sb, layer):
                """segment-sum over this core's edges; window-wise epilogue."""
                tile_off = 0    # global tile counter (canonical order)
                idx_off = [0, 0]
                for batch in g_batches:
                    tb = int(sum(g_nt[w, h] for w in batch for h in (0, 1)))
                    gb = gpool.tile([P, GBW, D], dt.float32, tag="gb")
                    # gathers: per (half, windows-of-batch)
                    t_cursor = 0
                    for h in (0, 1):
                        k = int(sum(g_nt[w, h] for w in batch))
                        if k == 0:
                            continue
                        nidx = k * P
                        idx_sb = g_idx0_sb if h == 0 else g_idx1_sb
                        c0 = idx_off[h] // 16
                        cols = nidx // 16
                        src = src_full[h * half:(h + 1) * half, :]
                        nc.gpsimd.dma_gather(
                            gb[:, t_cursor:t_cursor + k, :],
                            src, idx_sb[:, c0:c0 + cols],
                            num_idxs=nidx, num_idxs_reg=nidx,
                            elem_size=D, elem_step=D,
                            single_packet=(nidx <= 1024))
                        idx_off[h] += nidx
                        t_cursor += k
                    # per-window segment matmuls
                    # tile order inside gb: h0:(w0 tiles, w1 tiles), h1:(w0, w1)
                    w_tiles = {w: [] for w in batch}
                    tcur = 0
                    for h in (0, 1):
                        for w in batch:
                            for _ in range(int(g_nt[w, h])):
                                w_tiles[w].append(tcur)
                                tcur += 1
                    for w in batch:
                        acc = ppool.tile([P, P], dt.float32, tag="acc")
                        tl = w_tiles[w]
                        for i, t in enumerate(tl):
                            gt = tile_off + t
                            sel = selpool.tile([P, P], dt.float32, tag="sel")
                            nc.vector.tensor_scalar(
                                sel[:], iota_f[:],
                                g_meta_sb[:, gt, 0:1],
                                g_meta_sb[:, gt, 1:2],
                                op0=OP.is_equal, op1=OP.mult)
                            mm(acc[:], lhsT=gb[:, t, :], rhs=sel[:],
                               start=(i == 0), stop=(i == len(tl) - 1))
                        rows = slice(w * P, (w + 1) * P)
                        xT = wpool.tile([P, P], dt.float32, tag="xT")
                        nc.scalar.activation(xT[:], acc[:], AF.Identity,
                                             bias=bias_sb[:])
                        tp2 = ppool.tile([P, P], dt.float32, tag="tp")
                        nc.tensor.transpose(out=tp2[:], in_=xT[:], identity=ident[:])
                        x_rm = wpool.tile([P, P], dt.float32, tag="x_rm")
                        nc.scalar.activation(x_rm[:], tp2[:], AF.Copy)
                        ccol = D if layer == 1 else 2 * D
                        nc.sync.dma_start(cc_shard[rows, ccol:ccol + D], x_rm[:])
                        if layer == 1:
                            x1r_rm = wpool.tile([P, P], dt.float32, tag="x1r_rm")
                            nc.scalar.activation(x1r_rm[:], x_rm[:], AF.Relu)
                            nc.sync.dma_start(x1r_stash[rows, :], x1r_rm[:])
                            x1rT = wpool.tile([P, P], dt.float32, tag="x1rT")
                            nc.scalar.activation(x1rT[:], xT[:], AF.Relu)
                            s2p = ppool.tile([P, D], dt.float32, tag="s1p")
                            nc.tensor.matmul(s2p[:], lhsT=x1rT[:], rhs=W2_sb[:])
                            s2t = wpool.tile([P, D], dt.float32, tag="s1t")
                            nc.scalar.activation(s2t[:], s2p[:], AF.Copy)
                            nc.sync.dma_start(s2_shard[rows, :], s2t[:])
                        else:
                            embt2 = wpool.tile([P, P], dt.float32, tag="emb_t")
                            nc.sync.dma_start(embt2[:], emb_shard[rows, :])
                            x1rt2 = wpool.tile([P, P], dt.float32, tag="x1r_rd")
                            nc.sync.dma_start(x1rt2[:], x1r_stash[rows, :])
                            o1 = wpool.tile([P, P], dt.float32, tag="o1")
                            nc.vector.tensor_scalar(o1[:], x1rt2[:], 0.5, None,
                                                    op0=OP.mult)
                            nc.vector.tensor_tensor(o1[:], o1[:], embt2[:], op=OP.add)
                            o2 = wpool.tile([P, P], dt.float32, tag="o2")
                            nc.vector.tensor_scalar(o2[:], x_rm[:], 1.0 / 3.0, None,
                                                    op0=OP.mult)
                            nc.vector.tensor_tensor(o2[:], o2[:], o1[:], op=OP.add)
                            nc.sync.dma_start(out_shard[rows, :], o2[:])
                    tile_off += tb

            if stop_phase >= 2:
                gcn_layer(s1_full, b1_sb, layer=1)
            if stop_phase >= 3:
                nc.gpsimd.collective_compute(
                    "AllGather", OP.bypass, replica_groups=rg,
                    ins=[s2_shard.ap().opt()], outs=[s2_full.ap().opt()])
            if stop_phase >= 4:
                gcn_layer(s2_full, b2_sb, layer=2)
            gcn_psum.__exit__(None, None, None)
            if stop_phase >= 5:
                nc.gpsimd.collective_compute(
                    "AllGather", OP.bypass, replica_groups=rg,
                    ins=[cc_shard.ap().opt()], outs=[cc_full.ap().opt()])
            loss_psum = tc.tile_pool(name="loss_psum", bufs=1, space="PSUM")
            ppool1 = loss_psum.__enter__()
            loss_psum2 = tc.tile_pool(name="loss_psum2", bufs=1, space="PSUM")
            ppool = loss_psum2.__enter__()

            # ============ loss: pooling + MLP + logits ============
            tile_off = 0
            l_batches_eff = l_batches if stop_phase >= 6 else []
            idx_off = [0, 0]
            for batch in l_batches_eff:
                tb = int(sum(l_nt[w, h] for w in batch for h in (0, 1)))
                gb = lgpool.tile([P, LBW, CD], dt.float32, tag="lgb")
                t_cursor = 0
                for h in (0, 1):
                    k = int(sum(l_nt[w, h] for w in batch))
                    if k == 0:
                        continue
                    nidx = k * P
                    idx_sb = l_idx0_sb if h == 0 else l_idx1_sb
                    c0 = idx_off[h] // 16
                    cols = nidx // 16
                    src = cc_full[h * half:(h + 1) * half, :]
                    nc.gpsimd.dma_gather(
                        gb[:, t_cursor:t_cursor + k, :],
                        src, idx_sb[:, c0:c0 + cols],
                        num_idxs=nidx, num_idxs_reg=nidx,
                        elem_size=CD, elem_step=CD,
                        single_packet=(nidx <= 1024))
                    idx_off[h] += nidx
                    t_cursor += k
                w_tiles = {w: [] for w in batch}
                tcur = 0
                for h in (0, 1):
                    for w in batch:
                        for _ in range(int(l_nt[w, h])):
                            w_tiles[w].append(tcur)
                            tcur += 1
                for w in batch:
                    tl = w_tiles[w]
                    pool_ps = []
                    for j in range(3):
                        plp = ppool1.tile([P, P], dt.float32, tag=f"plp{j}")
                        pool_ps.append(plp)
                    for i, t in enumerate(tl):
                        gt = tile_off + t
                        sel = selpool.tile([P, P], dt.float32, tag="sel")
                        nc.vector.tensor_scalar(
                            sel[:], iota_f[:],
                            l_meta_sb[:, gt, 0:1],
                            l_meta_sb[:, gt, 1:2],
                            op0=OP.is_equal, op1=OP.mult)
                        for j in range(3):
                            mm(pool_ps[j][:], lhsT=gb[:, t, j * D:(j + 1) * D],
                               rhs=sel[:], start=(i == 0), stop=(i == len(tl) - 1))
                    pooled = wpool.tile([P, 3 * D], dt.float32, tag="pooled")
                    for j in range(3):
                        nc.scalar.activation(pooled[:, j * D:(j + 1) * D],
                                             pool_ps[j][:], AF.Copy)
                    if stop_phase < 7:
                        continue
                    hid = wpool.tile([P, 2 * 3 * D], dt.float32, tag="hid")
                    for hh in range(2):
                        hp = ppool.tile([P, 3 * D], dt.float32, tag="hp")
                        mm(hp[:], lhsT=hw_sb[:, hh * P:(hh + 1) * P], rhs=pooled[:])
                        nc.scalar.activation(hid[:, hh * 3 * D:(hh + 1) * 3 * D],
                                             hp[:], AF.Relu, bias=hb_sb[:, hh:hh + 1])
                    lg_ps = ppool.tile([1, 3 * D], dt.float32, tag="lg_ps")
                    mm(lg_ps[:], lhsT=pw_sb[:, 0:1], rhs=hid[:, 0:3 * D],
                       start=True, stop=False)
                    mm(lg_ps[:], lhsT=pw_sb[:, 1:2], rhs=hid[:, 3 * D:6 * D],
                       start=False, stop=True)
                    strip = spool.tile([1, 3 * D], dt.float32, tag="strip")
                    nc.scalar.activation(strip[:], lg_ps[:], AF.Identity,
                                         bias=pb_sb[:])
                    nc.sync.dma_start(logits_dram[w:w + 1, :], strip[:])
                tile_off += tb

            # ============ bce reduce ============
            Lt = wpool.tile([P, LG_T], dt.float32, tag="Lt")
            if stop_phase >= 8:
                nc.sync.dma_start(
                    Lt[:], logits_dram.ap().rearrange("a b -> (a b)")
                    .rearrange("(p t) -> p t", p=P))
            else:
                nc.vector.memset(Lt[:], 0.0)
            ns_sb = wpool.tile([P, LG_T], dt.float32, tag="ns")
            nc.sync.dma_start(ns_sb[:], negsign[:, :])
            lw_sb = wpool.tile([P, LG_T], dt.float32, tag="lw")
            nc.sync.dma_start(lw_sb[:], lweight[:, :])
            nc.vector.tensor_tensor(Lt[:], Lt[:], ns_sb[:], op=OP.mult)
            # softplus(m) = relu(m) + log1p(exp(-|m|)), with exp via exp2
            # round-and-poly and log1p via 2*atanh(u/(u+2)) series (no
            # Exp/Ln in this toolchain's ACT tables).
            LOG2E = 1.4426950408889634
            MAGIC = 1.5 * 2 ** 23
            EC = [1.0, 0.6931471805599453, 0.24022650695910072,
                  0.05550410866482158, 0.009618129107628477,
                  0.0013333558146428443]
            m_ = Lt
            t_ = wpool.tile([P, LG_T], dt.float32, tag="t_")
            nc.vector.tensor_scalar(t_[:], m_[:], -1.0, None, op0=OP.mult)
            nc.vector.tensor_tensor(t_[:], t_[:], m_[:], op=OP.min)   # -|m|
            nc.vector.tensor_scalar(t_[:], t_[:], -60.0, None, op0=OP.max)
            z_ = wpool.tile([P, LG_T], dt.float32, tag="z_")
            nc.vector.tensor_scalar(z_[:], t_[:], LOG2E, MAGIC,
                                    op0=OP.mult, op1=OP.add)          # z+magic
            nf = wpool.tile([P, LG_T], dt.float32, tag="nf")
            nc.vector.tensor_scalar(nf[:], z_[:], MAGIC, None, op0=OP.subtract)
            r_ = wpool.tile([P, LG_T], dt.float32, tag="r_")
            nc.vector.tensor_scalar(r_[:], t_[:], LOG2E, None, op0=OP.mult)
            nc.vector.tensor_tensor(r_[:], r_[:], nf[:], op=OP.subtract)
            # 2^n built exactly via value-convert: bits(2^n) = (n+127)*2^23,
            # and f32->int32 value conversion of (n+127)*2^23 yields those
            # bits; bitcast back to f32 gives 2^n with no int ALU ops.
            q_ = wpool.tile([P, LG_T], dt.float32, tag="q_")
            nc.vector.tensor_scalar(q_[:], nf[:], 127.0, float(2 ** 23),
                                    op0=OP.add, op1=OP.mult)
            qi = wpool.tile([P, LG_T], dt.int32, tag="qi")
            nc.vector.tensor_copy(qi[:], q_[:])
            ac = wpool.tile([P, LG_T], dt.float32, tag="ac")
            nc.vector.tensor_scalar(ac[:], r_[:], EC[5], EC[4],
                                    op0=OP.mult, op1=OP.add)
            for k in (3, 2, 1, 0):
                nc.vector.tensor_tensor(ac[:], ac[:], r_[:], op=OP.mult)
                nc.vector.tensor_scalar(ac[:], ac[:], EC[k], None, op0=OP.add)
            u_ = wpool.tile([P, LG_T], dt.float32, tag="u_")
            nc.vector.tensor_tensor(u_[:], ac[:], qi[:].bitcast(dt.float32),
                                    op=OP.mult)
            d_ = wpool.tile([P, LG_T], dt.float32, tag="d_")
            nc.vector.tensor_scalar(d_[:], u_[:], 2.0, None, op0=OP.add)
            rc = wpool.tile([P, LG_T], dt.float32, tag="rc")
            nc.vector.reciprocal(rc[:], d_[:])
            s_ = wpool.tile([P, LG_T], dt.float32, tag="s_")
            nc.vector.tensor_tensor(s_[:], u_[:], rc[:], op=OP.mult)
            y_ = wpool.tile([P, LG_T], dt.float32, tag="y_")
            nc.vector.tensor_tensor(y_[:], s_[:], s_[:], op=OP.mult)
            nc.vector.tensor_scalar(ac[:], y_[:], 1.0 / 9.0, 1.0 / 7.0,
                                    op0=OP.mult, op1=OP.add)
            for ck in (1.0 / 5.0, 1.0 / 3.0, 1.0):
                nc.vector.tensor_tensor(ac[:], ac[:], y_[:], op=OP.mult)
                nc.vector.tensor_scalar(ac[:], ac[:], ck, None, op0=OP.add)
            nc.vector.tensor_tensor(ac[:], ac[:], s_[:], op=OP.mult)
            nc.vector.tensor_scalar(ac[:], ac[:], 2.0, None, op0=OP.mult)
            sp = wpool.tile([P, LG_T], dt.float32, tag="sp")
            nc.scalar.activation(sp[:], m_[:], AF.Relu)
            nc.vector.tensor_tensor(sp[:], sp[:], ac[:], op=OP.add)
            nc.vector.tensor_tensor(sp[:], sp[:], lw_sb[:], op=OP.mult)
            csum = spool.tile([P, 1], dt.float32, tag="csum")
            nc.vector.reduce_sum(csum[:], sp[:], axis=mybir.AxisListType.X)
            tot = ppool.tile([1, 1], dt.float32, tag="tot")
            nc.tensor.matmul(tot[:], lhsT=ones_col[:], rhs=csum[:])
            ls = spool.tile([1, 1], dt.float32, tag="ls")
            nc.scalar.activation(ls[:], tot[:], AF.Copy)
            nc.sync.dma_start(loss_part[:, :], ls[:])
            loss_psum2.__exit__(None, None, None)
            loss_psum.__exit__(None, None, None)

    nc.compile()
    return nc


# ---------------------------------------------------------------------------
# Host orchestration
# ---------------------------------------------------------------------------

def _prepare(inputs, n=N, ncores=NCORES, gbw=2, lbw=1):
    """Host prep: shard + bucket + pad; returns (cfg, in_maps)."""
    edge_row = np.asarray(inputs["edge_row"]).astype(np.int64)
    edge_col = np.asarray(inputs["edge_col"]).astype(np.int64)
    edge_val = np.asarray(inputs["edge_val"]).astype(np.float32)
    pos = np.asarray(inputs["pos"]).astype(np.int64)
    neg = np.asarray(inputs["neg"]).astype(np.int64)
    emb = np.asarray(inputs["emb"]).astype(np.float32)
    kh = pos.shape[1]
    n_hyper = pos.shape[0] + neg.shape[0]
    wpc, rpc, npad, half, hpc_real, lwin, hpc = _derive(n, ncores, n_hyper)

    # ---- GCN edges, bucketed per destination core ----
    core_of = edge_row // rpc
    g_groups = []
    for c in range(ncores):
        m = core_of == c
        g_groups.append(_plan_segments(edge_row[m] - c * rpc, edge_col[m],
                                       edge_val[m], wpc, half, ncores))
    g_nt, g_batches = _pad_tiles(g_groups, wpc, gbw)

    # ---- loss hyperedges (pos then neg), data-parallel shard ----
    hyper = np.concatenate([pos, neg], axis=0)       # [n_hyper, kh]
    l_groups = []
    for c in range(ncores):
        lo, hi = c * hpc_real, min((c + 1) * hpc_real, n_hyper)
        rows = hyper[lo:hi]                           # [m, kh]
        mloc = np.repeat(np.arange(hi - lo), kh)      # local hyperedge id
        nodes = rows.reshape(-1)
        val = np.full(len(nodes), 1.0 / kh, np.float32)
        l_groups.append(_plan_segments(mloc, nodes, val, lwin, half, ncores))
    l_nt, l_batches = _pad_tiles(l_groups, lwin, lbw)

    # ---- per-core streams ----
    CD = 3 * D
    LG = lwin * CD
    LG_T = (LG + P - 1) // P
    in_maps = []
    g_c0 = g_c1 = l_c0 = l_c1 = None
    for c in range(ncores):
        gi0, gi1, gmeta = _emit_streams(g_groups[c], g_nt, g_batches)
        li0, li1, lmeta = _emit_streams(l_groups[c], l_nt, l_batches)
        g_c0, g_c1 = len(gi0) // 16, len(gi1) // 16
        l_c0, l_c1 = len(li0) // 16, len(li1) // 16

        # bce sign/weight tables in logits layout flat = w*CD + j*D + hmod,
        # reloaded as [P, LG_T] partition-major (flat = p*LG_T + t).
        nsv = np.zeros(LG_T * P, np.float32)
        lwv = np.zeros(LG_T * P, np.float32)
        lo = c * hpc_real
        for w in range(lwin):
            for j in range(3):
                base = w * CD + j * D
                hm = np.arange(P)
                hloc = w * P + hm
                valid = hloc < min(hpc_real, n_hyper - lo)
                gid = lo + hloc
                sign = np.where(gid < pos.shape[0], 1.0, -1.0)
                nsv[base:base + P] = -sign
                lwv[base:base + P] = np.where(valid, 1.0 / n_hyper, 0.0)
        in_map = {
            "emb_shard": _pad_rows(emb[c * rpc:(c + 1) * rpc], rpc),
            "W1": np.asarray(inputs["gc1_w"], np.float32),
            "b1": np.asarray(inputs["gc1_b"], np.float32).reshape(D, 1),
            "W2": np.asarray(inputs["gc2_w"], np.float32),
            "b2": np.asarray(inputs["gc2_b"], np.float32).reshape(D, 1),
            "h_w": np.asarray(inputs["h_w"], np.float32),
            "h_b": np.asarray(inputs["h_b"], np.float32).reshape(NH, 1),
            "p_w": np.asarray(inputs["p_w"], np.float32).reshape(NH, 1),
            "p_b": np.asarray(inputs["p_b"], np.float32).reshape(1, 1),
            "g_idx0": _wrap_idx(gi0), "g_idx1": _wrap_idx(gi1), "g_meta": gmeta,
            "l_idx0": _wrap_idx(li0), "l_idx1": _wrap_idx(li1), "l_meta": lmeta,
            "negsign": nsv.reshape(P, LG_T),
            "lweight": lwv.reshape(P, LG_T),
        }
        in_maps.append(in_map)

    cfg = dict(n=n, ncores=ncores, wpc=wpc, rpc=rpc, npad=npad, half=half,
               lwin=lwin, g_nt=g_nt, g_batches=g_batches,
               l_nt=l_nt, l_batches=l_batches,
               g_cols0=g_c0, g_cols1=g_c1, l_cols0=l_c0, l_cols1=l_c1)
    return cfg, in_maps


def _pad_rows(a, rows):
    if a.shape[0] == rows:
        return np.ascontiguousarray(a, np.float32)
    out = np.zeros((rows, a.shape[1]), np.float32)
    out[:a.shape[0]] = a
    return out


_CACHE = {}
RUN_KWARGS = {}      # test harness can set e.g. {"trace": True}
LAST_RESULT = [None]  # test harness reads profile/exec time from here
BUILD_OVERRIDES = {}  # test harness knobs (e.g. {"use_f32r": True})


def kernel(**inputs):
    from concourse import bass_utils
    cfg, in_maps = _prepare(inputs)
    cfg.update(BUILD_OVERRIDES)
    key = (cfg["wpc"], cfg["lwin"], tuple(cfg["g_nt"].ravel()),
           tuple(cfg["l_nt"].ravel()), tuple(sorted(BUILD_OVERRIDES.items())))
    if key not in _CACHE:
        _CACHE[key] = _build_program(cfg)
    nc = _CACHE[key]
    r = bass_utils.run_bass_kernel_spmd(
        nc, in_maps, core_ids=list(range(cfg["ncores"])), **RUN_KWARGS)
    LAST_RESULT[0] = r
    res = r.results
    out = np.concatenate([res[c]["out_shard"] for c in range(cfg["ncores"])],
                         axis=0)[:cfg["n"]]
    loss = np.float32(sum(float(res[c]["loss_part"][0, 0])
                          for c in range(cfg["ncores"])))
    return out, loss
